# revision 21
# baseline (speedup 1.0000x reference)
"""DVAE GNN message-passing kernel for 8 Trainium2 NeuronCores.

Data parallel over batch B=2048 -> 256 graphs/core (2 tiles of 128). Each core
runs the full 20-step topological scan with all weights replicated.

Math (per sample b, step v in 0..19, Hfwd starts at 0):
  gated_u = sigmoid(Wg @ [H_u, e_u] + bg) * (Wm @ [H_u, e_u])
  Hin_v   = sum_u adj[b,u,v] * gated_u      (u >= v rows give a constant
            contribution, precomputed on host and DMA'd as the slot seed)
  H_v     = GRUCell(x_v, Hin_v)
  mu,lv   = W1 @ H_19 + b1, W2 @ H_19 + b2

Device design notes:
  - Batch-major activations [128b, feat]; matmuls use the (transposed)
    activation as stationary and weights as moving, outputs land batch-major.
  - Per-step message inputs live in 40 persistent bf16 "slot" accumulators
    (one per (tile, step), ones-column at col HS baked in by the host).
    Each gated vector G_v is scattered into future slots with fused
    per-partition-scalar MACs on DVE (bf16 2x) / Pool (deferred queue).
  - All PE transposes run with a bf16 identity (1 cyc/row); Hin^T / H^T
    tiles are bf16 stationaries against f32r moving weights.
  - Emission order interleaves the two batch tiles so the PE stays fed
    during the GRU elementwise phase (pstate ramp).
"""

import sys
import numpy as np

for _p in ("/opt/trn_rl_repo",):
    if _p not in sys.path:
        sys.path.insert(0, _p)

B, MAXN, NVT, HS, NZ = 2048, 20, 26, 501, 56
HS2 = HS + 1                  # 502 (even innermost counts for fp32r)
NVT_EFF = NVT + MAXN          # 46
XDIM = NVT_EFF + 1            # 47
XD = XDIM + 1                 # 48: + ones row
NCORES = 8
BS = B // NCORES              # 256 samples per core
RZ = 2 * HS                   # 1002

# k-chunking of the augmented hidden axis (501 rows + ones row = 502)
CH = [(0, 128), (128, 128), (256, 128), (384, 118)]

DVE_SCAT_CAP = 16  # max scatter MACs per step on DVE (rest deferred)


def _packf_layout():
    """Column layout (f32 elements) of the packed f32r static tensor."""
    ents = {}
    col = 0

    def put(name, nrows, ncols):
        nonlocal col
        ents[name] = (nrows, col, ncols)
        col += ncols

    put("xub", XD, MAXN * BS)            # X^T + ones row
    for i, (o, s) in enumerate(CH):
        put(f"wrzh{i}", s, 2 * HS2)
    for i, (o, s) in enumerate(CH):
        put(f"whn{i}", s, HS2)
    put("wrzx", XD, 2 * HS2)
    put("wxn", XD, HS2)
    for i, (o, s) in enumerate(CH):
        put(f"wg{i}", s, HS2)
    put("wgv", MAXN, HS2)
    for i, (o, s) in enumerate(CH):
        put(f"wm{i}", s, HS2)
    put("wmv", MAXN, HS2)
    put("eye20", MAXN, MAXN)
    for i, (o, s) in enumerate(CH):
        put(f"w12{i}", s, 2 * NZ)
    return ents, col


# bf16 pack layout: identity | adj scalars | slots (w-major)
NB_IDB = 0                      # [128, 128]
NB_ADJ = 128                    # adjgb0/adjgb1: [128, 400] each
NB_SLOT0 = NB_ADJ + 800         # slot (t, w) at NB_SLOT0 + (w*2+t)*HS2
NBCOLS = NB_SLOT0 + 40 * HS2
NB_SPLITW = 6                   # slots w < 6 arrive in the first bpack DMA


_PROG = None  # cached Bass program


def _build_program():
    import concourse.bass as bass
    import concourse.tile as tile
    from concourse import bacc, mybir

    f32 = mybir.dt.float32
    f32r = mybir.dt.float32r
    bf16 = mybir.dt.bfloat16
    AF = mybir.ActivationFunctionType
    OP = mybir.AluOpType

    nc = bacc.Bacc("TRN2", target_bir_lowering=False, debug=False)

    ents, ncolsf = _packf_layout()
    d_wpack = nc.dram_tensor("wpack", [128, ncolsf], f32r,
                             kind="ExternalInput").ap()
    d_bpack = nc.dram_tensor("bpack", [128, NBCOLS], bf16,
                             kind="ExternalInput").ap()
    d_out = nc.dram_tensor("out", [BS, 2 * NZ], f32, kind="ExternalOutput").ap()

    def mm(out, lhsT, rhs, start, stop):
        nc.tensor.matmul(out, lhsT, rhs, start=start, stop=stop)

    with tile.TileContext(nc) as tc:
        with (
            tc.tile_pool(name="statics", bufs=1) as sp,
            tc.tile_pool(name="gstore", bufs=1) as gp,
            tc.tile_pool(name="sb", bufs=1) as wp,
            tc.tile_pool(name="pp", bufs=1, space="PSUM") as pp,
        ):
            # staged DMAs: step-0-critical data first so compute starts early
            WPACK = sp.tile([128, ncolsf], f32r, tag="wpack", name="wpack")
            BPACK = sp.tile([128, NBCOLS], bf16, tag="bpack", name="bpack")
            cw = ents["wg0"][1]           # gated weights arrive second
            cb = NB_SLOT0 + 2 * NB_SPLITW * HS2
            nc.sync.dma_start(WPACK[:, :cw], d_wpack[:, :cw])
            nc.sync.dma_start(BPACK[:, :cb], d_bpack[:, :cb])
            nc.sync.dma_start(WPACK[:, cw:], d_wpack[:, cw:])
            nc.sync.dma_start(BPACK[:, cb:], d_bpack[:, cb:])

            def sl(name, dt=None):
                nr, c0, ncl = ents[name]
                ap = WPACK[0:nr, c0:c0 + ncl]
                return ap.bitcast(dt) if dt else ap

            XUB = sl("xub")
            WRZH = [sl(f"wrzh{i}") for i in range(4)]
            WHN = [sl(f"whn{i}") for i in range(4)]
            WRZX = sl("wrzx")
            WXN = sl("wxn")
            WG = [sl(f"wg{i}") for i in range(4)]
            WM = [sl(f"wm{i}") for i in range(4)]
            WGV, WMV, EYE = sl("wgv"), sl("wmv"), sl("eye20")
            W12 = [sl(f"w12{i}") for i in range(4)]
            ADJG = [BPACK[:, NB_ADJ + 400 * t:NB_ADJ + 400 * (t + 1)]
                    for t in range(2)]

            def SLOT(t, w):
                c = NB_SLOT0 + (w * 2 + t) * HS2
                return BPACK[:, c:c + HS2]

            IDB = BPACK[:, NB_IDB:NB_IDB + 128]

            # G storage: one bf16 tile per (vertex, batch-tile)
            Gt = [[gp.tile([128, HS2], bf16, tag=f"g{_u}_{_t}",
                           name=f"g{_u}_{_t}")
                   for _t in range(2)] for _u in range(MAXN - 1)]

            # SBUF work tiles (tags give fixed buffers; bufs chosen for
            # cross-step pipelining where needed)
            def wtile(tag, shape, dt, bufs, name):
                return wp.tile(shape, dt, tag=tag, bufs=bufs, name=name)

            # psum tiles: all padded to one full 2KB bank
            def ptile(tag, dt, bufs, name):
                pad = [128, 512] if dt == f32 else [128, 1024]
                return pp.tile([128, 512], dt, tag=tag, bufs=bufs,
                               padded_shape=pad, name=name)

            # ---- scatter queue state (python-side scheduling) ----
            pend = []  # list of (w, u, t) pairs not yet emitted

            def emit_mac(eng, u, w, t):
                sc = ADJG[t][:, u * MAXN + w:u * MAXN + w + 1]
                eng.scalar_tensor_tensor(SLOT(t, w), Gt[u][t][:, :], sc,
                                         SLOT(t, w), OP.mult, OP.add)

            hint = {}
            ht = {}
            gates = {}

            def phase_A(v, t):
                """xseeds, transpose acc, hint copy, rz/hn matmuls for tile t."""
                xsl = XUB[:, v * BS + t * 128:v * BS + (t + 1) * 128]
                rz0 = ptile(f"rz0", f32, 1, f"rz0_{v}_{t}")
                rz1 = ptile(f"rz1", f32, 1, f"rz1_{v}_{t}")
                inp = ptile(f"inp", f32, 1, f"inp_{v}_{t}")
                hnp = ptile(f"hnp", f32, 1, f"hnp_{v}_{t}")
                gates[t] = (rz0, rz1, inp, hnp)
                mm(rz0[:, :HS2], xsl, WRZX[:, 0:HS2], start=True, stop=False)
                mm(rz1[:, :HS2], xsl, WRZX[:, HS2:2 * HS2], start=True,
                   stop=False)
                mm(inp[:, :HS2], xsl, WXN[:, :], start=True, stop=True)
                # transpose Hin (slot v) -> psum, then copy to sbuf (f32r)
                tp = ptile("tp", bf16, 2, f"tpa_{v}_{t}")
                acc = SLOT(t, v)
                for i, (o, w) in enumerate(CH):
                    nc.tensor.transpose(tp[0:w, i * 128:(i + 1) * 128],
                                        acc[:, o:o + w], IDB)
                hi = wtile(f"hint{t}", [128, 512], f32r, 2, f"hint_{v}_{t}")
                hint[t] = hi
                nc.scalar.copy(hi[:, :], tp[:, :512])

                def hc(i):
                    return hi[0:CH[i][1], i * 128:(i + 1) * 128]

                for i in range(4):
                    mm(rz0[:, :HS2], hc(i), WRZH[i][:, 0:HS2],
                       start=False, stop=(i == 3))
                for i in range(4):
                    mm(rz1[:, :HS2], hc(i), WRZH[i][:, HS2:2 * HS2],
                       start=False, stop=(i == 3))
                for i in range(4):
                    mm(hnp[:, :HS2], hc(i), WHN[i][:, :],
                       start=(i == 0), stop=(i == 3))

            def gru_front(v, t):
                """sigmoids + tanh input for tile t (Act/DVE)."""
                rz0, rz1, inp, hnp = gates[t]
                r = wtile(f"r{t}", [128, HS2], bf16, 1, f"r_{v}_{t}")
                z = wtile(f"z{t}", [128, HS2], bf16, 1, f"z_{v}_{t}")
                nc.scalar.activation(r[:, :HS], rz0[:, :HS], AF.Sigmoid)
                nc.scalar.activation(z[:, :HS], rz1[:, :HS], AF.Sigmoid)
                tmp = wtile(f"tmp{t}", [128, HS], f32, 1, f"tmp_{v}_{t}")
                nc.vector.tensor_tensor(tmp[:, :], r[:, :HS], hnp[:, :HS],
                                        OP.mult)
                nc.vector.tensor_tensor(tmp[:, :], tmp[:, :], inp[:, :HS],
                                        OP.add)
                return z, tmp

            def gru_back(v, t, z, n):
                """post-tanh GRU ops -> h tile (bf16); t0 on DVE, t1 on Pool
                (all-SBUF operands, keeps DVE free for the scatter MACs)."""
                eng = nc.vector if t == 0 else nc.gpsimd
                d = wtile(f"d{t}", [128, HS], bf16, 1, f"d_{v}_{t}")
                eng.tensor_tensor(d[:, :], SLOT(t, v)[:, :HS], n[:, :],
                                  OP.subtract)
                e = wtile(f"e{t}", [128, HS], bf16, 1, f"e_{v}_{t}")
                eng.tensor_tensor(e[:, :], d[:, :], z[:, :HS], OP.mult)
                h = wtile(f"h{t}", [128, HS2], bf16, 2, f"h_{v}_{t}")
                nc.gpsimd.memset(h[:, HS:HS2], 1.0)
                eng.tensor_tensor(h[:, :HS], e[:, :], n[:, :], OP.add)
                return h

            def phase_B_pe(v, t, h):
                """transpose h, ht copy, zp/mp matmuls for tile t."""
                tp = ptile("tp", bf16, 2, f"tpb_{v}_{t}")
                for i, (o, w) in enumerate(CH):
                    nc.tensor.transpose(tp[0:w, i * 128:(i + 1) * 128],
                                        h[:, o:o + w], IDB)
                hb = wtile(f"ht{t}", [128, 512], f32r, 2, f"ht_{v}_{t}")
                ht[t] = hb
                nc.scalar.copy(hb[:, :], tp[:, :512])

                def hc(i):
                    return hb[0:CH[i][1], i * 128:(i + 1) * 128]

                if v < MAXN - 1:
                    vsel = EYE[:, v:v + 1].broadcast_to([MAXN, 128])
                    zp = ptile("zp", f32, 1, f"zp_{v}_{t}")
                    mp = ptile("mp", f32, 1, f"mp_{v}_{t}")
                    mm(zp[:, :HS2], vsel, WGV[:, :], start=True, stop=False)
                    for i in range(4):
                        mm(zp[:, :HS2], hc(i), WG[i][:, :],
                           start=False, stop=(i == 3))
                    mm(mp[:, :HS2], vsel, WMV[:, :], start=True, stop=False)
                    for i in range(4):
                        mm(mp[:, :HS2], hc(i), WM[i][:, :],
                           start=False, stop=(i == 3))
                    return zp, mp
                return None, None

            def make_G(v, t, zp, mp):
                sg = wtile(f"sg{t}", [128, HS2], bf16, 1, f"sg_{v}_{t}")
                nc.scalar.activation(sg[:, :], zp[:, :HS2], AF.Sigmoid)
                nc.vector.tensor_tensor(Gt[v][t][:, :], sg[:, :],
                                        mp[:, :HS2], OP.mult)
                # critical MAC into the next step's slot
                emit_mac(nc.vector, v, v + 1, t)

            # ================= main loop =================
            for v in range(MAXN):
                phase_A(v, 0)
                z0, tmp0 = gru_front(v, 0)
                phase_A(v, 1)
                n0 = wtile("n0", [128, HS], bf16, 1, f"n_{v}_0")
                nc.scalar.activation(n0[:, :], tmp0[:, :], AF.Tanh)
                h0 = gru_back(v, 0, z0, n0)
                z1, tmp1 = gru_front(v, 1)
                zp0, mp0 = phase_B_pe(v, 0, h0)
                n1 = wtile("n1", [128, HS], bf16, 1, f"n_{v}_1")
                nc.scalar.activation(n1[:, :], tmp1[:, :], AF.Tanh)
                h1 = gru_back(v, 1, z1, n1)
                if v < MAXN - 1:
                    make_G(v, 0, zp0, mp0)
                zp1, mp1 = phase_B_pe(v, 1, h1)
                if v < MAXN - 1:
                    make_G(v, 1, zp1, mp1)
                    # queue far-future scatter for this step's G
                    for w in range(v + 2, MAXN):
                        pend.append((w, v, 0))
                        pend.append((w, v, 1))
                    # DVE drain: mandatory (w == v+1 targets) + budget
                    pend.sort()
                    ndve = 0
                    keep = []
                    for (w, u, t) in pend:
                        if w == v + 2 or ndve < DVE_SCAT_CAP:
                            emit_mac(nc.vector, u, w, t)
                            ndve += 1
                        else:
                            keep.append((w, u, t))
                    pend = keep

            # ---- readout from ht tiles of v=19 ----
            for t in range(2):
                op = ptile("zp", f32, 1, f"op_{t}")
                hb = ht[t]
                for i in range(4):
                    mm(op[:, :2 * NZ], hb[0:CH[i][1], i * 128:(i + 1) * 128],
                       W12[i][:, :], start=(i == 0), stop=(i == 3))
                ob = wtile(f"ob{t}", [128, 2 * NZ], f32, 1, f"ob_{t}")
                nc.scalar.copy(ob[:, :], op[:, :2 * NZ])
                nc.sync.dma_start(d_out[t * 128:(t + 1) * 128, :], ob[:, :])

    nc.compile()
    return nc


def _host_prep(types, feats, adj, Wg, bg, Wm, W_ih, b_ih, W_hh, b_hh, W1, b1,
               W2, b2):
    """Build per-core input maps (numpy only)."""
    import ml_dtypes
    bf16 = ml_dtypes.bfloat16
    f = np.float32
    types = np.asarray(types).astype(np.int64)
    feats = np.asarray(feats, dtype=f)
    adj = np.asarray(adj, dtype=f)
    Wg, bg, Wm = np.asarray(Wg, f), np.asarray(bg, f), np.asarray(Wm, f)
    W_ih, b_ih = np.asarray(W_ih, f), np.asarray(b_ih, f)
    W_hh, b_hh = np.asarray(W_hh, f), np.asarray(b_hh, f)
    W1, b1 = np.asarray(W1, f), np.asarray(b1, f)
    W2, b2 = np.asarray(W2, f), np.asarray(b2, f)

    bsz = types.shape[0]
    bs = bsz // NCORES

    # X^T with ones row: [48, MAXN*bs] per core
    X = np.zeros((bsz, MAXN, XD), dtype=f)
    onehot = np.eye(NVT_EFF, dtype=f)[types.reshape(-1) % NVT_EFF]
    X[:, :, :NVT_EFF] = onehot.reshape(bsz, MAXN, NVT_EFF)
    X[:, :, NVT_EFF] = feats
    X[:, :, XDIM] = 1.0

    # constant gated vectors c_u for zero hidden state
    zg = 1.0 / (1.0 + np.exp(-(bg[None, :] + Wg[:, HS:].T)))   # [20, 501]
    C = (zg * Wm[:, HS:].T).astype(f)
    # Hin constant part for every (sample, step): sum_{u>=w} adj[b,u,w]*C[u]
    umask = (np.arange(MAXN)[:, None] >= np.arange(MAXN)[None, :]).astype(f)
    hconst = np.einsum("buw,uh->bwh", adj * umask[None, :, :], C)  # [B,20,501]

    def aug(wT, brow):
        return np.concatenate([wT, brow[None, :]], axis=0).astype(f)

    def pad_rz(a):          # [s, 1002] -> [s, 1004] with per-gate 502 halves
        o = np.zeros((a.shape[0], 2 * HS2), dtype=f)
        o[:, :HS] = a[:, :HS]
        o[:, HS2:HS2 + HS] = a[:, HS:]
        return o

    def pad_h(a):           # [s, 501] -> [s, 502]
        o = np.zeros((a.shape[0], HS2), dtype=f)
        o[:, :HS] = a
        return o

    wrzh = pad_rz(aug(W_hh[:RZ].T, b_hh[:RZ]))
    whn = pad_h(aug(W_hh[RZ:].T, b_hh[RZ:]))
    wrzx = pad_rz(aug(W_ih[:RZ].T, b_ih[:RZ]))
    wxn = pad_h(aug(W_ih[RZ:].T, b_ih[RZ:]))
    wg = pad_h(np.concatenate([Wg[:, :HS].T, bg[None, :]], axis=0).astype(f))
    wgv = pad_h(np.ascontiguousarray(Wg[:, HS:].T))
    wm = pad_h(np.concatenate([Wm[:, :HS].T, np.zeros((1, HS), f)], axis=0))
    wmv = pad_h(np.ascontiguousarray(Wm[:, HS:].T))
    eye20 = np.eye(MAXN, dtype=f)
    w12 = np.concatenate([np.concatenate([W1.T, W2.T], axis=1),
                          np.concatenate([b1, b2])[None, :]], axis=0).astype(f)

    ents, ncolsf = _packf_layout()

    def place(pack, name, arr):
        nr, c0, ncl = ents[name]
        assert arr.shape == (nr, ncl), (name, arr.shape, (nr, ncl))
        pack[0:nr, c0:c0 + ncl] = arr

    in_maps = []
    for c in range(NCORES):
        sli = slice(c * bs, (c + 1) * bs)
        Xc = X[sli]                                   # [bs, 20, 48]
        xt = Xc.transpose(2, 1, 0).reshape(XD, MAXN * bs)
        adjc = adj[sli]                               # [bs, 20, 20]

        pack = np.zeros((128, ncolsf), dtype=f)
        place(pack, "xub", xt)
        for i, (o, s) in enumerate(CH):
            place(pack, f"wrzh{i}", wrzh[o:o + s])
            place(pack, f"whn{i}", whn[o:o + s])
            place(pack, f"w12{i}", w12[o:o + s])
            place(pack, f"wg{i}", wg[o:o + s])
            place(pack, f"wm{i}", wm[o:o + s])
        place(pack, "wrzx", wrzx)
        place(pack, "wxn", wxn)
        place(pack, "wgv", wgv)
        place(pack, "wmv", wmv)
        place(pack, "eye20", eye20)

        bpack = np.zeros((128, NBCOLS), dtype=bf16)
        bpack[:, NB_IDB:NB_IDB + 128] = np.eye(128, dtype=f)
        adjg = adjc.reshape(bs, MAXN * MAXN)
        bpack[:, NB_ADJ:NB_ADJ + 400] = adjg[:128]
        bpack[:, NB_ADJ + 400:NB_ADJ + 800] = adjg[128:]
        hcc = hconst[sli]                             # [bs, 20, 501]
        for t in range(2):
            for w in range(MAXN):
                col = NB_SLOT0 + (w * 2 + t) * HS2
                bpack[:, col:col + HS] = hcc[t * 128:(t + 1) * 128, w, :]
                bpack[:, col + HS] = 1.0

        in_maps.append(dict(wpack=pack, bpack=bpack))
    return in_maps


def _get_prog():
    global _PROG
    if _PROG is None:
        _PROG = _build_program()
    return _PROG


def kernel(**inputs):
    from concourse.bass_utils import run_bass_kernel_spmd
    nc = _get_prog()
    in_maps = _host_prep(**inputs)
    res = run_bass_kernel_spmd(nc, in_maps, core_ids=list(range(NCORES)))
    out = np.concatenate([r["out"] for r in res.results], axis=0)
    mu = np.ascontiguousarray(out[:, :NZ])
    logvar = np.ascontiguousarray(out[:, NZ:])
    return mu, logvar


# revision 22
# speedup vs baseline: 1.2020x; 1.2020x over previous
"""DVAE GNN message-passing kernel for 8 Trainium2 NeuronCores.

Data parallel over batch B=2048 -> 256 graphs/core (2 tiles of 128). Each core
runs the full 20-step topological scan with all weights replicated.

Math (per sample b, step v in 0..19, Hfwd starts at 0):
  gated_u = sigmoid(Wg @ [H_u, e_u] + bg) * (Wm @ [H_u, e_u])
  Hin_v   = sum_u adj[b,u,v] * gated_u      (u >= v rows give a constant
            contribution, precomputed on host and DMA'd as the slot seed)
  H_v     = GRUCell(x_v, Hin_v)
  mu,lv   = W1 @ H_19 + b1, W2 @ H_19 + b2

Device design notes:
  - Batch-major activations [128b, feat]; matmuls use the (transposed)
    activation as stationary and weights as moving, outputs land batch-major.
  - Per-step message inputs live in 40 persistent bf16 "slot" accumulators
    (one per (tile, step), ones-column at col HS baked in by the host).
    Each gated vector G_v is scattered into future slots with fused
    per-partition-scalar MACs on DVE (bf16 2x) / Pool (deferred queue).
  - All PE transposes run with a bf16 identity (1 cyc/row); Hin^T / H^T
    tiles are bf16 stationaries against f32r moving weights.
  - Emission order interleaves the two batch tiles so the PE stays fed
    during the GRU elementwise phase (pstate ramp).
"""

import sys
import numpy as np

for _p in ("/opt/trn_rl_repo",):
    if _p not in sys.path:
        sys.path.insert(0, _p)

B, MAXN, NVT, HS, NZ = 2048, 20, 26, 501, 56
HS2 = HS + 1                  # 502 (even innermost counts for fp32r)
NVT_EFF = NVT + MAXN          # 46
XDIM = NVT_EFF + 1            # 47
XD = XDIM + 1                 # 48: + ones row
NCORES = 8
BS = B // NCORES              # 256 samples per core
RZ = 2 * HS                   # 1002

# k-chunking of the augmented hidden axis (501 rows + ones row = 502)
CH = [(0, 128), (128, 128), (256, 128), (384, 118)]

DVE_SCAT_CAP = 16  # max scatter MACs per step on DVE (rest deferred)


def _packf_layout():
    """Column layout (f32 elements) of the packed f32r static tensor."""
    ents = {}
    col = 0

    def put(name, nrows, ncols):
        nonlocal col
        ents[name] = (nrows, col, ncols)
        col += ncols

    put("xub", XD, MAXN * BS)            # X^T + ones row
    for i, (o, s) in enumerate(CH):
        put(f"wrzh{i}", s, 2 * HS2)
    for i, (o, s) in enumerate(CH):
        put(f"whn{i}", s, HS2)
    put("wrzx", XD, 2 * HS2)
    put("wxn", XD, HS2)
    for i, (o, s) in enumerate(CH):
        put(f"wg{i}", s, HS2)
    put("wgv", MAXN, HS2)
    for i, (o, s) in enumerate(CH):
        put(f"wm{i}", s, HS2)
    put("wmv", MAXN, HS2)
    put("eye20", MAXN, MAXN)
    for i, (o, s) in enumerate(CH):
        put(f"w12{i}", s, 2 * NZ)
    return ents, col


# bf16 pack layout: identity | adj scalars | slots (w-major)
NB_IDB = 0                      # [128, 128]
NB_ADJ = 128                    # adjgb0/adjgb1: [128, 400] each
NB_SLOT0 = NB_ADJ + 800         # slot (t, w) at NB_SLOT0 + (w*2+t)*HS2
NBCOLS = NB_SLOT0 + 40 * HS2
NB_SPLITW = 6                   # slots w < 6 arrive in the first bpack DMA


_PROG = None  # cached Bass program


def _build_program():
    import concourse.bass as bass
    import concourse.tile as tile
    from concourse import bacc, mybir

    f32 = mybir.dt.float32
    f32r = mybir.dt.float32r
    bf16 = mybir.dt.bfloat16
    AF = mybir.ActivationFunctionType
    OP = mybir.AluOpType

    nc = bacc.Bacc("TRN2", target_bir_lowering=False, debug=False)

    ents, ncolsf = _packf_layout()
    d_wpack = nc.dram_tensor("wpack", [128, ncolsf], f32r,
                             kind="ExternalInput").ap()
    d_bpack = nc.dram_tensor("bpack", [128, NBCOLS], bf16,
                             kind="ExternalInput").ap()
    d_out = nc.dram_tensor("out", [BS, 2 * NZ], f32, kind="ExternalOutput").ap()

    def mm(out, lhsT, rhs, start, stop):
        nc.tensor.matmul(out, lhsT, rhs, start=start, stop=stop)

    with tile.TileContext(nc) as tc:
        with (
            tc.tile_pool(name="statics", bufs=1) as sp,
            tc.tile_pool(name="gstore", bufs=1) as gp,
            tc.tile_pool(name="sb", bufs=1) as wp,
            tc.tile_pool(name="pp", bufs=1, space="PSUM") as pp,
        ):
            # staged DMAs: step-0-critical data first so compute starts early
            WPACK = sp.tile([128, ncolsf], f32r, tag="wpack", name="wpack")
            BPACK = sp.tile([128, NBCOLS], bf16, tag="bpack", name="bpack")
            cw = ents["wg0"][1]           # gated weights arrive second
            cb = NB_SLOT0 + 2 * NB_SPLITW * HS2
            nc.sync.dma_start(WPACK[:, :cw], d_wpack[:, :cw])
            nc.sync.dma_start(BPACK[:, :cb], d_bpack[:, :cb])
            nc.sync.dma_start(WPACK[:, cw:], d_wpack[:, cw:])
            nc.sync.dma_start(BPACK[:, cb:], d_bpack[:, cb:])

            def sl(name, dt=None):
                nr, c0, ncl = ents[name]
                ap = WPACK[0:nr, c0:c0 + ncl]
                return ap.bitcast(dt) if dt else ap

            XUB = sl("xub")
            WRZH = [sl(f"wrzh{i}") for i in range(4)]
            WHN = [sl(f"whn{i}") for i in range(4)]
            WRZX = sl("wrzx")
            WXN = sl("wxn")
            WG = [sl(f"wg{i}") for i in range(4)]
            WM = [sl(f"wm{i}") for i in range(4)]
            WGV, WMV, EYE = sl("wgv"), sl("wmv"), sl("eye20")
            W12 = [sl(f"w12{i}") for i in range(4)]
            ADJG = [BPACK[:, NB_ADJ + 400 * t:NB_ADJ + 400 * (t + 1)]
                    for t in range(2)]

            def SLOT(t, w):
                c = NB_SLOT0 + (w * 2 + t) * HS2
                return BPACK[:, c:c + HS2]

            IDB = BPACK[:, NB_IDB:NB_IDB + 128]

            # G storage: one bf16 tile per (vertex, batch-tile)
            Gt = [[gp.tile([128, HS2], bf16, tag=f"g{_u}_{_t}",
                           name=f"g{_u}_{_t}")
                   for _t in range(2)] for _u in range(MAXN - 1)]

            # SBUF work tiles (tags give fixed buffers; bufs chosen for
            # cross-step pipelining where needed)
            def wtile(tag, shape, dt, bufs, name):
                return wp.tile(shape, dt, tag=tag, bufs=bufs, name=name)

            # psum tiles: all padded to one full 2KB bank
            def ptile(tag, dt, bufs, name):
                pad = [128, 512] if dt == f32 else [128, 1024]
                return pp.tile([128, 512], dt, tag=tag, bufs=bufs,
                               padded_shape=pad, name=name)

            # ---- scatter queue state (python-side scheduling) ----
            pend = []  # list of (w, u, t) pairs not yet emitted

            def emit_mac(eng, u, w, t):
                sc = ADJG[t][:, u * MAXN + w:u * MAXN + w + 1]
                eng.scalar_tensor_tensor(SLOT(t, w), Gt[u][t][:, :], sc,
                                         SLOT(t, w), OP.mult, OP.add)

            hint = {}
            ht = {}
            gates = {}

            def phase_A(v, t):
                """xseeds, transpose acc, hint copy, rz/hn matmuls for tile t."""
                xsl = XUB[:, v * BS + t * 128:v * BS + (t + 1) * 128]
                rz0 = ptile(f"rz0", f32, 1, f"rz0_{v}_{t}")
                rz1 = ptile(f"rz1", f32, 1, f"rz1_{v}_{t}")
                inp = ptile(f"inp", f32, 1, f"inp_{v}_{t}")
                hnp = ptile(f"hnp", f32, 1, f"hnp_{v}_{t}")
                gates[t] = (rz0, rz1, inp, hnp)
                mm(rz0[:, :HS2], xsl, WRZX[:, 0:HS2], start=True, stop=False)
                mm(rz1[:, :HS2], xsl, WRZX[:, HS2:2 * HS2], start=True,
                   stop=False)
                mm(inp[:, :HS2], xsl, WXN[:, :], start=True, stop=True)
                # transpose Hin (slot v) -> psum, then copy to sbuf (f32r)
                tp = ptile("tp", bf16, 2, f"tpa_{v}_{t}")
                acc = SLOT(t, v)
                for i, (o, w) in enumerate(CH):
                    nc.tensor.transpose(tp[0:w, i * 128:(i + 1) * 128],
                                        acc[:, o:o + w], IDB)
                hi = wtile(f"hint{t}", [128, 512], f32r, 2, f"hint_{v}_{t}")
                hint[t] = hi
                nc.scalar.copy(hi[:, :], tp[:, :512])

                def hc(i):
                    return hi[0:CH[i][1], i * 128:(i + 1) * 128]

                for i in range(4):
                    mm(rz0[:, :HS2], hc(i), WRZH[i][:, 0:HS2],
                       start=False, stop=(i == 3))
                for i in range(4):
                    mm(rz1[:, :HS2], hc(i), WRZH[i][:, HS2:2 * HS2],
                       start=False, stop=(i == 3))
                for i in range(4):
                    mm(hnp[:, :HS2], hc(i), WHN[i][:, :],
                       start=(i == 0), stop=(i == 3))

            def gru_front(v, t):
                """sigmoids + tanh input for tile t (Act/DVE)."""
                rz0, rz1, inp, hnp = gates[t]
                r = wtile(f"r{t}", [128, HS2], bf16, 1, f"r_{v}_{t}")
                z = wtile(f"z{t}", [128, HS2], bf16, 1, f"z_{v}_{t}")
                nc.scalar.activation(r[:, :HS], rz0[:, :HS], AF.Sigmoid)
                nc.scalar.activation(z[:, :HS], rz1[:, :HS], AF.Sigmoid)
                tmp = wtile(f"tmp{t}", [128, HS], f32, 1, f"tmp_{v}_{t}")
                nc.vector.tensor_tensor(tmp[:, :], r[:, :HS], hnp[:, :HS],
                                        OP.mult)
                nc.vector.tensor_tensor(tmp[:, :], tmp[:, :], inp[:, :HS],
                                        OP.add)
                return z, tmp

            def gru_back(v, t, z, n):
                """post-tanh GRU ops -> h tile (bf16); t0 on DVE, t1 on Pool
                (all-SBUF operands, keeps DVE free for the scatter MACs)."""
                eng = nc.vector if t == 0 else nc.gpsimd
                d = wtile(f"d{t}", [128, HS], bf16, 1, f"d_{v}_{t}")
                eng.tensor_tensor(d[:, :], SLOT(t, v)[:, :HS], n[:, :],
                                  OP.subtract)
                e = wtile(f"e{t}", [128, HS], bf16, 1, f"e_{v}_{t}")
                eng.tensor_tensor(e[:, :], d[:, :], z[:, :HS], OP.mult)
                h = wtile(f"h{t}", [128, HS2], bf16, 2, f"h_{v}_{t}")
                nc.gpsimd.memset(h[:, HS:HS2], 1.0)
                eng.tensor_tensor(h[:, :HS], e[:, :], n[:, :], OP.add)
                return h

            def phase_B_pe(v, t, h):
                """transpose h, ht copy, zp/mp matmuls for tile t."""
                tp = ptile("tp", bf16, 2, f"tpb_{v}_{t}")
                for i, (o, w) in enumerate(CH):
                    nc.tensor.transpose(tp[0:w, i * 128:(i + 1) * 128],
                                        h[:, o:o + w], IDB)
                hb = wtile(f"ht{t}", [128, 512], f32r, 2, f"ht_{v}_{t}")
                ht[t] = hb
                nc.scalar.copy(hb[:, :], tp[:, :512])

                def hc(i):
                    return hb[0:CH[i][1], i * 128:(i + 1) * 128]

                if v < MAXN - 1:
                    vsel = EYE[:, v:v + 1].broadcast_to([MAXN, 128])
                    zp = ptile("zp", f32, 1, f"zp_{v}_{t}")
                    mp = ptile("mp", f32, 1, f"mp_{v}_{t}")
                    mm(zp[:, :HS2], vsel, WGV[:, :], start=True, stop=False)
                    for i in range(4):
                        mm(zp[:, :HS2], hc(i), WG[i][:, :],
                           start=False, stop=(i == 3))
                    mm(mp[:, :HS2], vsel, WMV[:, :], start=True, stop=False)
                    for i in range(4):
                        mm(mp[:, :HS2], hc(i), WM[i][:, :],
                           start=False, stop=(i == 3))
                    return zp, mp
                return None, None

            def make_G(v, t, zp, mp):
                sg = wtile(f"sg{t}", [128, HS2], bf16, 1, f"sg_{v}_{t}")
                nc.scalar.activation(sg[:, :], zp[:, :HS2], AF.Sigmoid)
                nc.vector.tensor_tensor(Gt[v][t][:, :], sg[:, :],
                                        mp[:, :HS2], OP.mult)
                # critical MAC into the next step's slot
                emit_mac(nc.vector, v, v + 1, t)

            # ================= main loop =================
            # Software-pipelined: step v+1's A-phase (tile 0) + GRU front are
            # emitted before step v's B-phase (tile 1), so next-step critical
            # DVE/Act work jumps the queue ahead of deferrable scatter MACs.
            phase_A(0, 0)
            zt0 = gru_front(0, 0)
            for v in range(MAXN):
                z0, tmp0 = zt0
                phase_A(v, 1)
                n0 = wtile("n0", [128, HS], bf16, 1, f"n_{v}_0")
                nc.scalar.activation(n0[:, :], tmp0[:, :], AF.Tanh)
                h0 = gru_back(v, 0, z0, n0)
                z1, tmp1 = gru_front(v, 1)
                zp0, mp0 = phase_B_pe(v, 0, h0)
                n1 = wtile("n1", [128, HS], bf16, 1, f"n_{v}_1")
                nc.scalar.activation(n1[:, :], tmp1[:, :], AF.Tanh)
                h1 = gru_back(v, 1, z1, n1)
                if v < MAXN - 1:
                    make_G(v, 0, zp0, mp0)
                    phase_A(v + 1, 0)
                    zt0 = gru_front(v + 1, 0)
                zp1, mp1 = phase_B_pe(v, 1, h1)
                if v < MAXN - 1:
                    make_G(v, 1, zp1, mp1)
                    # queue far-future scatter for this step's G
                    for w in range(v + 2, MAXN):
                        pend.append((w, v, 0))
                        pend.append((w, v, 1))
                    # DVE drain: mandatory (w == v+2 targets) + budget
                    pend.sort()
                    ndve = 0
                    keep = []
                    for (w, u, t) in pend:
                        if w == v + 2 or ndve < DVE_SCAT_CAP:
                            emit_mac(nc.vector, u, w, t)
                            ndve += 1
                        else:
                            keep.append((w, u, t))
                    pend = keep

            # ---- readout from ht tiles of v=19 ----
            for t in range(2):
                op = ptile("zp", f32, 1, f"op_{t}")
                hb = ht[t]
                for i in range(4):
                    mm(op[:, :2 * NZ], hb[0:CH[i][1], i * 128:(i + 1) * 128],
                       W12[i][:, :], start=(i == 0), stop=(i == 3))
                ob = wtile(f"ob{t}", [128, 2 * NZ], f32, 1, f"ob_{t}")
                nc.scalar.copy(ob[:, :], op[:, :2 * NZ])
                nc.sync.dma_start(d_out[t * 128:(t + 1) * 128, :], ob[:, :])

    nc.compile()
    return nc


def _host_prep(types, feats, adj, Wg, bg, Wm, W_ih, b_ih, W_hh, b_hh, W1, b1,
               W2, b2):
    """Build per-core input maps (numpy only)."""
    import ml_dtypes
    bf16 = ml_dtypes.bfloat16
    f = np.float32
    types = np.asarray(types).astype(np.int64)
    feats = np.asarray(feats, dtype=f)
    adj = np.asarray(adj, dtype=f)
    Wg, bg, Wm = np.asarray(Wg, f), np.asarray(bg, f), np.asarray(Wm, f)
    W_ih, b_ih = np.asarray(W_ih, f), np.asarray(b_ih, f)
    W_hh, b_hh = np.asarray(W_hh, f), np.asarray(b_hh, f)
    W1, b1 = np.asarray(W1, f), np.asarray(b1, f)
    W2, b2 = np.asarray(W2, f), np.asarray(b2, f)

    bsz = types.shape[0]
    bs = bsz // NCORES

    # X^T with ones row: [48, MAXN*bs] per core
    X = np.zeros((bsz, MAXN, XD), dtype=f)
    onehot = np.eye(NVT_EFF, dtype=f)[types.reshape(-1) % NVT_EFF]
    X[:, :, :NVT_EFF] = onehot.reshape(bsz, MAXN, NVT_EFF)
    X[:, :, NVT_EFF] = feats
    X[:, :, XDIM] = 1.0

    # constant gated vectors c_u for zero hidden state
    zg = 1.0 / (1.0 + np.exp(-(bg[None, :] + Wg[:, HS:].T)))   # [20, 501]
    C = (zg * Wm[:, HS:].T).astype(f)
    # Hin constant part for every (sample, step): sum_{u>=w} adj[b,u,w]*C[u]
    umask = (np.arange(MAXN)[:, None] >= np.arange(MAXN)[None, :]).astype(f)
    hconst = np.einsum("buw,uh->bwh", adj * umask[None, :, :], C)  # [B,20,501]

    def aug(wT, brow):
        return np.concatenate([wT, brow[None, :]], axis=0).astype(f)

    def pad_rz(a):          # [s, 1002] -> [s, 1004] with per-gate 502 halves
        o = np.zeros((a.shape[0], 2 * HS2), dtype=f)
        o[:, :HS] = a[:, :HS]
        o[:, HS2:HS2 + HS] = a[:, HS:]
        return o

    def pad_h(a):           # [s, 501] -> [s, 502]
        o = np.zeros((a.shape[0], HS2), dtype=f)
        o[:, :HS] = a
        return o

    wrzh = pad_rz(aug(W_hh[:RZ].T, b_hh[:RZ]))
    whn = pad_h(aug(W_hh[RZ:].T, b_hh[RZ:]))
    wrzx = pad_rz(aug(W_ih[:RZ].T, b_ih[:RZ]))
    wxn = pad_h(aug(W_ih[RZ:].T, b_ih[RZ:]))
    wg = pad_h(np.concatenate([Wg[:, :HS].T, bg[None, :]], axis=0).astype(f))
    wgv = pad_h(np.ascontiguousarray(Wg[:, HS:].T))
    wm = pad_h(np.concatenate([Wm[:, :HS].T, np.zeros((1, HS), f)], axis=0))
    wmv = pad_h(np.ascontiguousarray(Wm[:, HS:].T))
    eye20 = np.eye(MAXN, dtype=f)
    w12 = np.concatenate([np.concatenate([W1.T, W2.T], axis=1),
                          np.concatenate([b1, b2])[None, :]], axis=0).astype(f)

    ents, ncolsf = _packf_layout()

    def place(pack, name, arr):
        nr, c0, ncl = ents[name]
        assert arr.shape == (nr, ncl), (name, arr.shape, (nr, ncl))
        pack[0:nr, c0:c0 + ncl] = arr

    in_maps = []
    for c in range(NCORES):
        sli = slice(c * bs, (c + 1) * bs)
        Xc = X[sli]                                   # [bs, 20, 48]
        xt = Xc.transpose(2, 1, 0).reshape(XD, MAXN * bs)
        adjc = adj[sli]                               # [bs, 20, 20]

        pack = np.zeros((128, ncolsf), dtype=f)
        place(pack, "xub", xt)
        for i, (o, s) in enumerate(CH):
            place(pack, f"wrzh{i}", wrzh[o:o + s])
            place(pack, f"whn{i}", whn[o:o + s])
            place(pack, f"w12{i}", w12[o:o + s])
            place(pack, f"wg{i}", wg[o:o + s])
            place(pack, f"wm{i}", wm[o:o + s])
        place(pack, "wrzx", wrzx)
        place(pack, "wxn", wxn)
        place(pack, "wgv", wgv)
        place(pack, "wmv", wmv)
        place(pack, "eye20", eye20)

        bpack = np.zeros((128, NBCOLS), dtype=bf16)
        bpack[:, NB_IDB:NB_IDB + 128] = np.eye(128, dtype=f)
        adjg = adjc.reshape(bs, MAXN * MAXN)
        bpack[:, NB_ADJ:NB_ADJ + 400] = adjg[:128]
        bpack[:, NB_ADJ + 400:NB_ADJ + 800] = adjg[128:]
        hcc = hconst[sli]                             # [bs, 20, 501]
        for t in range(2):
            for w in range(MAXN):
                col = NB_SLOT0 + (w * 2 + t) * HS2
                bpack[:, col:col + HS] = hcc[t * 128:(t + 1) * 128, w, :]
                bpack[:, col + HS] = 1.0

        in_maps.append(dict(wpack=pack, bpack=bpack))
    return in_maps


def _get_prog():
    global _PROG
    if _PROG is None:
        _PROG = _build_program()
    return _PROG


def kernel(**inputs):
    from concourse.bass_utils import run_bass_kernel_spmd
    nc = _get_prog()
    in_maps = _host_prep(**inputs)
    res = run_bass_kernel_spmd(nc, in_maps, core_ids=list(range(NCORES)))
    out = np.concatenate([r["out"] for r in res.results], axis=0)
    mu = np.ascontiguousarray(out[:, :NZ])
    logvar = np.ascontiguousarray(out[:, NZ:])
    return mu, logvar
